# revision 10
# baseline (speedup 1.0000x reference)
"""Multi-head causal attention (B=2, S=2048, D=1024, H=16) on 8 TRN2 NeuronCores.

Sharding: core c handles batch b = c//4 and head-group g = c%4 (4 heads, 256 dims).
Each core computes Q/K/V projections for its head group from x[b], runs causal
attention per head, and applies its 256 rows of Wo, producing a partial [S, D]
output. The host sums the 4 head-group partials per batch.

v3 design notes (per core), relative to the original bf16 kernel:
  - Optional fp8 (e4m3) DoubleRow path for the Q/K/V projections: weights are
    host-scaled by 16 (keeps them out of the fp8 denormal range); the scale
    cancels in softmax via the exp-scale fold (q,k both x16 -> logits x256 ->
    ACT scale = SCALE/256) and in O via the ones-column trick (ones col = 16
    so numerator and denominator both carry the 16x of v-hat). Out projection
    stays bf16 (fp8 error would not average down there).
  - A ~4us burst of dummy matmuls at t=0 warms the PE HAM clock gate so real
    work runs at 2.4 GHz as soon as the input DMAs land.
  - Pair-1 Q/K projections are enqueued in reversed chunk order with per-chunk
    markers, so pair-1 attention starts after draining only its own chunk's
    projections instead of the whole leftover queue (the original stalled
    ScalarE ~15us between the pairs).
  - Diagonal-block mask multiplies split between DVE (head 0) and the
    otherwise-idle GpSimd (head 1).
  - The final chunk's out-projection PSUM->SBUF copies run on ScalarE, which
    is idle after the last exp; earlier chunks use DVE.
  - Normalization keeps the original lane-parallel shape (denominator rows
    bounced through a DRAM scratch to reshape [2,512] -> [128,8] for the DVE
    reciprocal, then broadcast back with a stride-0 partition read): a direct
    [64,512] reciprocal costs 3.3us/op on DVE (per-lane serial depth).
"""

import os
from collections import deque

import ml_dtypes
import numpy as np

import concourse.bass as bass
import concourse.mybir as mybir
import concourse.tile as tile
from concourse.bass_utils import run_bass_kernel_spmd
from concourse.masks import make_upper_triangular

F32 = mybir.dt.float32
BF16 = mybir.dt.bfloat16
F8 = mybir.dt.float8e4
DRMODE = mybir.MatmulPerfMode.DoubleRow

B, S, D, H = 2, 2048, 1024, 16
HD = 64                     # head dim
GH = 4                      # heads per core
GC = GH * HD                # 256 projection cols per core
P = 128
KD = D // P                 # 8 contraction chunks for projections
KDH = KD // 2               # 4 DoubleRow k-pair chunks
NSB = S // P                # 16 seq blocks
CHW = 512                   # i-chunk width
NCH = S // CHW              # 4 i-chunks
SCALE = HD ** -0.5

USE_FP8 = False             # fp8 q/k-path error is j-coherent: unsafe (4e-2)
WSCALE = 16.0 if USE_FP8 else 1.0
ACT_SCALE = SCALE / (WSCALE * WSCALE)
VONE = WSCALE               # ones-column value (makes denom track v-hat scale)
XDT = F8 if USE_FP8 else BF16

V1 = HD + 1
NWARM = 38                  # dummy matmuls to warm the PE clock gate
# fillers per strip group, by (pair, chunk). Pairs interleave at chunk level
# (p0c0, p1c0, p0c1, ...), so each window pre-feeds the next chunk's
# projections plus the previous chunk's out-projection stream.
FILLN = {(0, 0): 6, (1, 0): 7, (0, 1): 6, (1, 1): 6,
         (0, 2): 5, (1, 2): 5, (0, 3): 1, (1, 3): 0}

_NC_CACHE = None
LAST_RESULTS = None         # BassKernelResults of the most recent run (for test.py)


class _Fillers:
    """Queue of small emission closures (1-2 engine ops each) drained between
    attention strip groups to keep the PE busy while ScalarE works through
    the exp stream. Markers let the consumer force-drain the prefix a
    dependent phase needs."""

    def __init__(self):
        self.q = deque()

    def add(self, fn):
        self.q.append(fn)

    def add_marker(self, key):
        self.q.append(key)

    def _emit_one(self):
        item = self.q.popleft()
        if callable(item):
            item()
            return None
        return item

    def step(self, n):
        done = 0
        while done < n and self.q:
            if self._emit_one() is None:
                done += 1

    def drain_until(self, key):
        while self.q:
            if self._emit_one() == key:
                return

    def drain(self):
        while self.q:
            self._emit_one()


def _emit_chunk_attention(tc, pair, c, pools, tensors, fillers,
                          emit_outproj):
    nc = tc.nc
    ps_sc, ps_pv, dpool, ppool, npool, opool = pools
    qT, kT, v_sb, oT, trimask = tensors

    if True:
        njb = 4 * c + 4
        pvacc0 = ps_pv.tile([V1, CHW], F32, tag="pv0", name="pvacc0")
        pvacc1 = ps_pv.tile([V1, CHW], F32, tag="pv1", name="pvacc1")
        pvacc = {0: pvacc0, 1: pvacc1}
        # strip tasks, heads interleaved so paired score matmuls are adjacent
        tasks = [(hp, jb) for jb in range(njb) for hp in (0, 1)]
        for g0 in range(0, len(tasks), 2):
            group = tasks[g0:g0 + 2]
            sc = ps_sc.tile([P, 2, CHW], F32, tag="sc")
            pt = ppool.tile([P, 2, CHW], BF16, tag="pt")
            for t, (hp, jb) in enumerate(group):
                bp = hp * HD
                tl = max(0, jb - 4 * c) * P
                nc.tensor.matmul(
                    sc[:, t, tl:],
                    kT[bp:bp + HD, pair, jb * P:(jb + 1) * P],
                    qT[bp:bp + HD, pair, c * CHW + tl:(c + 1) * CHW])
            tlg = max(0, group[0][1] - 4 * c) * P
            nc.scalar.activation(
                pt[:, :len(group), tlg:], sc[:, :len(group), tlg:],
                mybir.ActivationFunctionType.Exp, scale=ACT_SCALE)
            for t, (hp, jb) in enumerate(group):
                if jb >= 4 * c:               # diagonal block: causal mask
                    tl = (jb - 4 * c) * P
                    nc.vector.tensor_mul(
                        pt[:, t, tl:tl + P], pt[:, t, tl:tl + P], trimask)
            for t, (hp, jb) in enumerate(group):
                h = pair * 2 + hp
                tl = max(0, jb - 4 * c) * P
                nc.tensor.matmul(
                    pvacc[hp][:, tl:], v_sb[:, jb, h, :], pt[:, t, tl:],
                    start=(jb == 0), stop=(jb == njb - 1))
            fillers.step(FILLN[(pair, c)])

        # per-chunk normalize for both heads: copy num/denom out of PSUM,
        # lane-parallel reciprocal via a DRAM reshape bounce, broadcast, mul
        cs = slice(c * CHW, (c + 1) * CHW)
        dden = dpool.tile([2, CHW], F32, tag="dden")
        onums = {}
        for hp in (0, 1):
            onum = opool.tile([V1, CHW], F32, tag=f"on{hp}")
            nc.vector.tensor_copy(out=onum, in_=pvacc[hp])
            nc.sync.dma_start(
                out=dden[hp:hp + 1, :], in_=onum[HD:V1, :])
            onums[hp] = onum
        nel = 2 * CHW // P                    # 8 elems/lane
        rv = npool.tile([P, nel], F32, tag="recp")
        nc.sync.dma_start(out=rv, in_=bass.AP(
            tensor=dden.tensor, offset=dden.offset, ap=[[nel, P], [1, nel]]))
        nc.vector.reciprocal(out=rv, in_=rv)
        drec = dpool.tile([2, CHW], F32, tag="drec")
        nc.sync.dma_start(out=bass.AP(
            tensor=drec.tensor, offset=drec.offset,
            ap=[[nel, P], [1, nel]]), in_=rv)
        for hp in (0, 1):
            bcr = npool.tile([HD, CHW], F32, tag=f"bcr{hp}")
            nc.sync.dma_start(out=bcr, in_=bass.AP(
                tensor=drec.tensor, offset=drec.offset + hp * CHW,
                ap=[[0, HD], [1, CHW]]))
            if hp == 0:
                nc.vector.tensor_mul(
                    oT[0:HD, pair, cs], onums[hp][0:HD, :], bcr)
            else:
                tmp = npool.tile([HD, CHW], BF16, tag="otmp")
                nc.vector.tensor_mul(tmp, onums[hp][0:HD, :], bcr)
                nc.sync.dma_start(out=oT[HD:P, pair, cs], in_=tmp)
        if emit_outproj is not None:
            emit_outproj(c)


def _emit(tc):
    nc = tc.nc
    xT = nc.dram_tensor("xT", [D, S], XDT, kind="ExternalInput")
    wq = nc.dram_tensor("wq", [D, GC], XDT, kind="ExternalInput")
    wk = nc.dram_tensor("wk", [D, GC], XDT, kind="ExternalInput")
    wv = nc.dram_tensor("wv", [D, GC], XDT, kind="ExternalInput")
    wo = nc.dram_tensor("wo", [GC, D], BF16, kind="ExternalInput")
    y = nc.dram_tensor("y", [S, D], F32, kind="ExternalOutput")

    xT_t = xT[:].rearrange("(o p) s -> p o s", p=P)      # [128, 8, S]
    wq_t = wq[:].rearrange("(o p) c -> p o c", p=P)      # [128, 8, 256]
    wk_t = wk[:].rearrange("(o p) c -> p o c", p=P)
    wv_t = wv[:].rearrange("(o p) c -> p o c", p=P)
    wo_t = wo[:].rearrange("(o p) n -> p o n", p=P)      # [128, 2, 1024]

    from contextlib import ExitStack

    with ExitStack() as top:
        persist = top.enter_context(tc.tile_pool(name="persist", bufs=1))

        trimask = persist.tile([P, P], BF16)             # 1.0 where j<=i else 0
        make_upper_triangular(nc, trimask, val=1.0, diag=True)

        wq_sb = persist.tile([P, KD, GC], XDT)
        wk_sb = persist.tile([P, KD, GC], XDT)
        wv_sb = persist.tile([P, KD, GC], XDT)
        wo_sb = persist.tile([P, 2, D], BF16)
        xfull = persist.tile([P, KD, S], XDT)
        warm = persist.tile([P, P], BF16)

        qT = persist.tile([P, 2, S], BF16)               # [pair-cols, pair, seq]
        kT = persist.tile([P, 2, S], BF16)
        v_sb = persist.tile([P, NSB, GH, V1], BF16)      # ones col appended
        oT = persist.tile([P, 2, S], BF16)

        ps_sc = top.enter_context(
            tc.tile_pool(name="ps_sc", bufs=2, space="PSUM"))
        ps_pv = top.enter_context(
            tc.tile_pool(name="ps_pv", bufs=1, space="PSUM"))
        ps_fill = top.enter_context(
            tc.tile_pool(name="ps_fill", bufs=2, space="PSUM"))
        dpool = top.enter_context(
            tc.tile_pool(name="dscr", bufs=4, space="DRAM"))
        ppool = top.enter_context(tc.tile_pool(name="pstrip", bufs=3))
        npool = top.enter_context(tc.tile_pool(name="norm", bufs=4))
        opool = top.enter_context(tc.tile_pool(name="onum", bufs=2))
        ypool = top.enter_context(tc.tile_pool(name="ystage", bufs=2))
        pools = (ps_sc, ps_pv, dpool, ppool, npool, opool)

        # ---- PE warmup: dense dummy matmuls flip the HAM clock gate while
        # the input DMAs land ----
        nc.vector.memset(warm, 0.0)
        wps = ps_fill.tile([P, CHW], F32, tag="fill", name="warmps")
        for _ in range(NWARM):
            nc.tensor.matmul(wps[:, 0:P], warm, warm, start=True, stop=True)

        nc.gpsimd.memset(v_sb[:, :, :, HD:V1], VONE)

        # ---- input DMAs: first-needed slices first, split across the two
        # HWDGE queues ----
        for k2 in range(0, KD, 2):
            nc.sync.dma_start(
                out=xfull[:, k2:k2 + 2, 0:CHW], in_=xT_t[:, k2:k2 + 2, 0:CHW])
            nc.scalar.dma_start(
                out=wq_sb[:, k2:k2 + 2, :], in_=wq_t[:, k2:k2 + 2, :])
            nc.scalar.dma_start(
                out=wk_sb[:, k2:k2 + 2, :], in_=wk_t[:, k2:k2 + 2, :])
        nc.scalar.dma_start(out=wv_sb, in_=wv_t)
        for ch in range(1, NCH):
            for k2 in range(0, KD, 2):
                eng = nc.sync if (ch + k2 // 2) % 2 == 0 else nc.scalar
                eng.dma_start(
                    out=xfull[:, k2:k2 + 2, ch * CHW:(ch + 1) * CHW],
                    in_=xT_t[:, k2:k2 + 2, ch * CHW:(ch + 1) * CHW])
        nc.gpsimd.dma_start(out=wo_sb, in_=wo_t)

        tensors = (qT, kT, v_sb, oT, trimask)

        f = _Fillers()

        def _proj_dr(which, pair_, ch):
            # which: 0=Q, 1=K; 4 DoubleRow matmuls + copy-out
            cell = {}
            w_sb = wq_sb if which == 0 else wk_sb
            dst = qT if which == 0 else kT

            def mm(t, cell=cell, ch=ch, w_sb=w_sb, pair_=pair_):
                if t == 0:
                    cell["p"] = ps_fill.tile(
                        [P, CHW], F32, tag="fill", name="fillqk")
                if USE_FP8:
                    nc.tensor.matmul(
                        cell["p"],
                        w_sb[:, 2 * t:2 * t + 2, pair_ * P:(pair_ + 1) * P],
                        xfull[:, 2 * t:2 * t + 2, ch * CHW:(ch + 1) * CHW],
                        start=(t == 0), stop=(t == KDH - 1),
                        perf_mode=DRMODE)
                else:
                    for u in (2 * t, 2 * t + 1):
                        nc.tensor.matmul(
                            cell["p"],
                            w_sb[:, u, pair_ * P:(pair_ + 1) * P],
                            xfull[:, u, ch * CHW:(ch + 1) * CHW],
                            start=(u == 0), stop=(u == KD - 1))

            def copy(cell=cell, ch=ch, dst=dst, pair_=pair_):
                nc.vector.tensor_copy(
                    out=dst[:, pair_, ch * CHW:(ch + 1) * CHW],
                    in_=cell["p"])

            for t in range(KDH):
                f.add(lambda t=t: mm(t))
            f.add(copy)

        def _v_dr(sb):
            cell = {}

            def mm(t, cell=cell, sb=sb):
                if t == 0:
                    cell["pv"] = ps_fill.tile(
                        [P, CHW], F32, tag="fill", name="fillpv")
                if USE_FP8:
                    nc.tensor.matmul(
                        cell["pv"][:, 0:GC],
                        xfull[:, 2 * t:2 * t + 2, sb * P:(sb + 1) * P],
                        wv_sb[:, 2 * t:2 * t + 2, :],
                        start=(t == 0), stop=(t == KDH - 1),
                        perf_mode=DRMODE)
                else:
                    for u in (2 * t, 2 * t + 1):
                        nc.tensor.matmul(
                            cell["pv"][:, 0:GC],
                            xfull[:, u, sb * P:(sb + 1) * P],
                            wv_sb[:, u, :],
                            start=(u == 0), stop=(u == KD - 1))

            def copy(cell=cell, sb=sb):
                nc.vector.tensor_copy(
                    out=v_sb[:, sb, :, 0:HD],
                    in_=cell["pv"][:, 0:GC].rearrange(
                        "p (h d) -> p h d", h=GH))

            for t in range(KDH):
                f.add(lambda t=t: mm(t))
            f.add(copy)

        # prerequisites in consumption order: per chunk, pair-0 Q/K + V
        # blocks, then pair-1 Q/K
        for ch in range(NCH):
            _proj_dr(0, 0, ch)
            _proj_dr(1, 0, ch)
            for s4 in range(CHW // P):
                _v_dr(ch * (CHW // P) + s4)
            f.add_marker(("pre0", ch))
            _proj_dr(0, 1, ch)
            _proj_dr(1, 1, ch)
            f.add_marker(("qk1", ch))

        def _outproj_chunk(c):
            for s4 in range(CHW // P):
                sb = c * (CHW // P) + s4
                cell = {}

                def alloc(cell=cell):
                    cell["ysb"] = ypool.tile(
                        [P, D], F32, tag="ysb", name="ysb")

                f.add(alloc)
                for nch in range(2):
                    def mm(gc, cell=cell, sb=sb, nch=nch):
                        if gc == 0:
                            cell["py"] = ps_fill.tile(
                                [P, CHW], F32, tag="fill", name="fillpy")
                        nc.tensor.matmul(
                            cell["py"], oT[:, gc, sb * P:(sb + 1) * P],
                            wo_sb[:, gc, nch * CHW:(nch + 1) * CHW],
                            start=(gc == 0), stop=(gc == 1))

                    def cp(cell=cell, nch=nch, c=c):
                        dst = cell["ysb"][:, nch * CHW:(nch + 1) * CHW]
                        if c == NCH - 1:
                            nc.scalar.copy(out=dst, in_=cell["py"])
                        else:
                            nc.vector.tensor_copy(out=dst, in_=cell["py"])

                    f.add(lambda mm=mm: mm(0))
                    f.add(lambda mm=mm: mm(1))
                    f.add(cp)

                def out_dma(cell=cell, sb=sb):
                    nc.sync.dma_start(
                        out=y[sb * P:(sb + 1) * P, :], in_=cell["ysb"])

                f.add(out_dma)

        for c in range(NCH):
            f.drain_until(("pre0", c))
            _emit_chunk_attention(tc, 0, c, pools, tensors, f, None)
            f.drain_until(("qk1", c))
            _emit_chunk_attention(tc, 1, c, pools, tensors, f, _outproj_chunk)
        f.drain()


def _fix_instruction_waits(nc):
    """Some lowered ISA structs (fp32r matmul LDW, DMA pseudo) carry at most
    one sync wait. Normalize: hoist excess waits onto NoOps inserted
    immediately before the instruction in the scheduled stream (same engine,
    so program order preserves the wait semantics)."""
    fixed = 0
    for blk in nc.m.functions[0].blocks:
        insts = blk.instructions
        idx = 0
        while idx < len(insts):
            inst = insts[idx]
            si = getattr(inst, "sync_info", None)
            if si is not None and len(si.on_wait) > 1:
                waits = list(si.on_wait)
                for j, wt in enumerate(waits[:-1]):
                    nop = mybir.InstNoOp(
                        name=f"I-wfix{fixed}-{j}-{inst.name}",
                        engine=inst.engine,
                        sync_info=mybir.SyncInfo(on_wait=[wt], on_update=[]))
                    insts.insert(idx, nop)
                    idx += 1
                inst.sync_info = mybir.SyncInfo(
                    on_wait=[waits[-1]], on_update=list(si.on_update))
                fixed += 1
            idx += 1
    return fixed


def _build():
    global _NC_CACHE
    if _NC_CACHE is None:
        nc = bass.Bass()
        with tile.TileContext(nc) as tc:
            _emit(tc)
        _fix_instruction_waits(nc)
        _NC_CACHE = nc
    return _NC_CACHE


def kernel(x, Wq, Wkv, Wo):
    global LAST_RESULTS
    x = np.asarray(x, dtype=np.float32)
    Wq = np.asarray(Wq, dtype=np.float32)
    Wkv = np.asarray(Wkv, dtype=np.float32)
    Wo = np.asarray(Wo, dtype=np.float32)

    nc = _build()
    bf = ml_dtypes.bfloat16
    xdt = ml_dtypes.float8_e4m3 if USE_FP8 else bf
    in_maps = []
    for c in range(8):
        b, g = divmod(c, 4)
        cs = slice(GC * g, GC * (g + 1))
        in_maps.append({
            "xT": np.ascontiguousarray(x[b].T).astype(xdt),
            "wq": np.ascontiguousarray(Wq[:, cs] * WSCALE).astype(xdt),
            "wk": np.ascontiguousarray(
                Wkv[:, 0:D][:, cs] * WSCALE).astype(xdt),
            "wv": np.ascontiguousarray(
                Wkv[:, D:2 * D][:, cs] * WSCALE).astype(xdt),
            "wo": np.ascontiguousarray(Wo[cs, :]).astype(bf),
        })

    trace = os.environ.get("ATTN_KERNEL_TRACE", "0") == "1"
    res = run_bass_kernel_spmd(nc, in_maps, list(range(8)), trace=trace)
    LAST_RESULTS = res

    out = np.zeros((B, S, D), dtype=np.float32)
    for c in range(8):
        b = c // 4
        out[b] += res.results[c]["y"]
    return out


if __name__ == "__main__":
    rng = np.random.default_rng(0)
    s = 1.0 / np.sqrt(D)
    inputs = {
        "x": rng.standard_normal((B, S, D), dtype=np.float32),
        "Wq": rng.standard_normal((D, D), dtype=np.float32) * s,
        "Wkv": rng.standard_normal((D, 2 * D), dtype=np.float32) * s,
        "Wo": rng.standard_normal((D, D), dtype=np.float32) * s,
    }
    out = kernel(**inputs)
    print("out", out.shape, out.dtype, float(np.abs(out).mean()))


# revision 14
# speedup vs baseline: 1.0213x; 1.0213x over previous
"""Multi-head causal attention (B=2, S=2048, D=1024, H=16) on 8 TRN2 NeuronCores.

Sharding: core c handles batch b = c//4 and head-group g = c%4 (4 heads, 256 dims).
Each core computes Q/K/V projections for its head group from x[b], runs causal
attention per head, and applies its 256 rows of Wo, producing a partial [S, D]
output. The host sums the 4 head-group partials per batch.

v3 design notes (per core), relative to the original bf16 kernel:
  - Optional fp8 (e4m3) DoubleRow path for the Q/K/V projections: weights are
    host-scaled by 16 (keeps them out of the fp8 denormal range); the scale
    cancels in softmax via the exp-scale fold (q,k both x16 -> logits x256 ->
    ACT scale = SCALE/256) and in O via the ones-column trick (ones col = 16
    so numerator and denominator both carry the 16x of v-hat). Out projection
    stays bf16 (fp8 error would not average down there).
  - A ~4us burst of dummy matmuls at t=0 warms the PE HAM clock gate so real
    work runs at 2.4 GHz as soon as the input DMAs land.
  - Pair-1 Q/K projections are enqueued in reversed chunk order with per-chunk
    markers, so pair-1 attention starts after draining only its own chunk's
    projections instead of the whole leftover queue (the original stalled
    ScalarE ~15us between the pairs).
  - Diagonal-block mask multiplies split between DVE (head 0) and the
    otherwise-idle GpSimd (head 1).
  - The final chunk's out-projection PSUM->SBUF copies run on ScalarE, which
    is idle after the last exp; earlier chunks use DVE.
  - Normalization keeps the original lane-parallel shape (denominator rows
    bounced through a DRAM scratch to reshape [2,512] -> [128,8] for the DVE
    reciprocal, then broadcast back with a stride-0 partition read): a direct
    [64,512] reciprocal costs 3.3us/op on DVE (per-lane serial depth).
"""

import os
from collections import deque

import ml_dtypes
import numpy as np

import concourse.bass as bass
import concourse.mybir as mybir
import concourse.tile as tile
from concourse.bass_utils import run_bass_kernel_spmd
from concourse.masks import make_upper_triangular

F32 = mybir.dt.float32
BF16 = mybir.dt.bfloat16
F8 = mybir.dt.float8e4
DRMODE = mybir.MatmulPerfMode.DoubleRow

B, S, D, H = 2, 2048, 1024, 16
HD = 64                     # head dim
GH = 4                      # heads per core
GC = GH * HD                # 256 projection cols per core
P = 128
KD = D // P                 # 8 contraction chunks for projections
KDH = KD // 2               # 4 DoubleRow k-pair chunks
NSB = S // P                # 16 seq blocks
CHW = 512                   # i-chunk width
NCH = S // CHW              # 4 i-chunks
SCALE = HD ** -0.5

USE_FP8 = False             # fp8 q/k-path error is j-coherent: unsafe (4e-2)
WSCALE = 16.0 if USE_FP8 else 1.0
ACT_SCALE = SCALE / (WSCALE * WSCALE)
# softmax shift: P-hat = exp(logit - 3) keeps the fp8 P-hat below the TRN
# fp8e4 NaN band (241..448) for ~8.5-sigma logits; softmax is shift-invariant
EXP_BIAS = -3.0
VONE = WSCALE               # ones-column value (makes denom track v-hat scale)
XDT = F8 if USE_FP8 else BF16

V1 = HD + 1
VP = 80                     # padded v' row (fp8 DoubleRow needs 16B-aligned
                            # k-tile stride: 4 heads x 80 bytes = 320)
NWARM = 38                  # dummy matmuls to warm the PE clock gate
# fillers per strip group, by (pair, chunk). Pairs interleave at chunk level
# (p0c0, p1c0, p0c1, ...), so each window pre-feeds the next chunk's
# projections plus the previous chunk's out-projection stream.
FILLN = {(0, 0): 6, (1, 0): 7, (0, 1): 6, (1, 1): 6,
         (0, 2): 5, (1, 2): 5, (0, 3): 1, (1, 3): 0}

_NC_CACHE = None
LAST_RESULTS = None         # BassKernelResults of the most recent run (for test.py)


class _Fillers:
    """Queue of small emission closures (1-2 engine ops each) drained between
    attention strip groups to keep the PE busy while ScalarE works through
    the exp stream. Markers let the consumer force-drain the prefix a
    dependent phase needs."""

    def __init__(self):
        self.q = deque()

    def add(self, fn):
        self.q.append(fn)

    def add_marker(self, key):
        self.q.append(key)

    def _emit_one(self):
        item = self.q.popleft()
        if callable(item):
            item()
            return None
        return item

    def step(self, n):
        done = 0
        while done < n and self.q:
            if self._emit_one() is None:
                done += 1

    def drain_until(self, key):
        while self.q:
            if self._emit_one() == key:
                return

    def drain(self):
        while self.q:
            self._emit_one()


def _emit_chunk_attention(tc, pair, c, pools, tensors, fillers,
                          emit_outproj):
    nc = tc.nc
    ps_sc, ps_pv, dpool, ppool, npool, opool = pools
    qT, kT, v_sb, oT, trimask = tensors

    if True:
        njb = 4 * c + 4
        pvacc0 = ps_pv.tile([V1, CHW], F32, tag="pv0", name="pvacc0")
        pvacc1 = ps_pv.tile([V1, CHW], F32, tag="pv1", name="pvacc1")
        pvacc = {0: pvacc0, 1: pvacc1}
        # strip tasks, heads interleaved so paired score matmuls are adjacent
        tasks = [(hp, jb) for jb in range(njb) for hp in (0, 1)]
        for g0 in range(0, len(tasks), 2):
            group = tasks[g0:g0 + 2]
            sc = ps_sc.tile([P, 2, CHW], F32, tag="sc")
            pt = ppool.tile([P, 2, CHW], BF16, tag="pt")
            for t, (hp, jb) in enumerate(group):
                bp = hp * HD
                tl = max(0, jb - 4 * c) * P
                nc.tensor.matmul(
                    sc[:, t, tl:],
                    kT[bp:bp + HD, pair, jb * P:(jb + 1) * P],
                    qT[bp:bp + HD, pair, c * CHW + tl:(c + 1) * CHW])
            tlg = max(0, group[0][1] - 4 * c) * P
            nc.scalar.activation(
                pt[:, :len(group), tlg:], sc[:, :len(group), tlg:],
                mybir.ActivationFunctionType.Exp, scale=ACT_SCALE)
            for t, (hp, jb) in enumerate(group):
                if jb >= 4 * c:               # diagonal block: causal mask
                    tl = (jb - 4 * c) * P
                    nc.vector.tensor_mul(
                        pt[:, t, tl:tl + P], pt[:, t, tl:tl + P], trimask)
            for t, (hp, jb) in enumerate(group):
                h = pair * 2 + hp
                tl = max(0, jb - 4 * c) * P
                nc.tensor.matmul(
                    pvacc[hp][:, tl:], v_sb[:, jb, h, :], pt[:, t, tl:],
                    start=(jb == 0), stop=(jb == njb - 1))
            fillers.step(FILLN[(pair, c)])

        # per-chunk normalize for both heads: copy num/denom out of PSUM,
        # lane-parallel reciprocal via a DRAM reshape bounce, broadcast, mul
        cs = slice(c * CHW, (c + 1) * CHW)
        dden = dpool.tile([2, CHW], F32, tag="dden")
        onums = {}
        for hp in (0, 1):
            onum = opool.tile([V1, CHW], F32, tag=f"on{hp}")
            nc.vector.tensor_copy(out=onum, in_=pvacc[hp])
            nc.sync.dma_start(
                out=dden[hp:hp + 1, :], in_=onum[HD:V1, :])
            onums[hp] = onum
        nel = 2 * CHW // P                    # 8 elems/lane
        rv = npool.tile([P, nel], F32, tag="recp")
        nc.sync.dma_start(out=rv, in_=bass.AP(
            tensor=dden.tensor, offset=dden.offset, ap=[[nel, P], [1, nel]]))
        nc.vector.reciprocal(out=rv, in_=rv)
        drec = dpool.tile([2, CHW], F32, tag="drec")
        nc.sync.dma_start(out=bass.AP(
            tensor=drec.tensor, offset=drec.offset,
            ap=[[nel, P], [1, nel]]), in_=rv)
        for hp in (0, 1):
            bcr = npool.tile([HD, CHW], F32, tag=f"bcr{hp}")
            nc.sync.dma_start(out=bcr, in_=bass.AP(
                tensor=drec.tensor, offset=drec.offset + hp * CHW,
                ap=[[0, HD], [1, CHW]]))
            if hp == 0:
                nc.vector.tensor_mul(
                    oT[0:HD, pair, cs], onums[hp][0:HD, :], bcr)
            else:
                tmp = npool.tile([HD, CHW], BF16, tag="otmp")
                nc.vector.tensor_mul(tmp, onums[hp][0:HD, :], bcr)
                nc.sync.dma_start(out=oT[HD:P, pair, cs], in_=tmp)
        if emit_outproj is not None:
            emit_outproj(c)


def _emit(tc):
    nc = tc.nc
    xT = nc.dram_tensor("xT", [D, S], XDT, kind="ExternalInput")
    wq = nc.dram_tensor("wq", [D, GC], XDT, kind="ExternalInput")
    wk = nc.dram_tensor("wk", [D, GC], XDT, kind="ExternalInput")
    wv = nc.dram_tensor("wv", [D, GC], XDT, kind="ExternalInput")
    wo = nc.dram_tensor("wo", [GC, D], BF16, kind="ExternalInput")
    y = nc.dram_tensor("y", [S, D], F32, kind="ExternalOutput")

    xT_t = xT[:].rearrange("(o p) s -> p o s", p=P)      # [128, 8, S]
    wq_t = wq[:].rearrange("(o p) c -> p o c", p=P)      # [128, 8, 256]
    wk_t = wk[:].rearrange("(o p) c -> p o c", p=P)
    wv_t = wv[:].rearrange("(o p) c -> p o c", p=P)
    wo_t = wo[:].rearrange("(o p) n -> p o n", p=P)      # [128, 2, 1024]

    from contextlib import ExitStack

    with ExitStack() as top:
        persist = top.enter_context(tc.tile_pool(name="persist", bufs=1))

        trimask = persist.tile([P, P], BF16)             # 1.0 where j<=i else 0
        make_upper_triangular(nc, trimask, val=1.0, diag=True)

        wq_sb = persist.tile([P, KD, GC], XDT)
        wk_sb = persist.tile([P, KD, GC], XDT)
        wv_sb = persist.tile([P, KD, GC], XDT)
        wo_sb = persist.tile([P, 2, D], BF16)
        xfull = persist.tile([P, KD, S], XDT)
        warm = persist.tile([P, P], BF16)

        qT = persist.tile([P, 2, S], BF16)               # [pair-cols, pair, seq]
        kT = persist.tile([P, 2, S], BF16)
        v_sb = persist.tile([P, NSB, GH, V1], BF16)      # ones col appended
        oT = persist.tile([P, 2, S], BF16)

        ps_sc = top.enter_context(
            tc.tile_pool(name="ps_sc", bufs=2, space="PSUM"))
        ps_pv = top.enter_context(
            tc.tile_pool(name="ps_pv", bufs=1, space="PSUM"))
        ps_fill = top.enter_context(
            tc.tile_pool(name="ps_fill", bufs=2, space="PSUM"))
        dpool = top.enter_context(
            tc.tile_pool(name="dscr", bufs=4, space="DRAM"))
        ppool = top.enter_context(tc.tile_pool(name="pstrip", bufs=3))
        npool = top.enter_context(tc.tile_pool(name="norm", bufs=4))
        opool = top.enter_context(tc.tile_pool(name="onum", bufs=2))
        ypool = top.enter_context(tc.tile_pool(name="ystage", bufs=2))
        pools = (ps_sc, ps_pv, dpool, ppool, npool, opool)

        # ---- PE warmup: dense dummy matmuls flip the HAM clock gate while
        # the input DMAs land ----
        nc.vector.memset(warm, 0.0)
        wps = ps_fill.tile([P, CHW], F32, tag="fill", name="warmps")
        for _ in range(NWARM):
            nc.tensor.matmul(wps[:, 0:P], warm, warm, start=True, stop=True)

        nc.gpsimd.memset(v_sb[:, :, :, HD:V1], VONE)  # fp8 ones

        # ---- input DMAs: first-needed slices first, split across the two
        # HWDGE queues ----
        for k in range(KD):
            nc.sync.dma_start(
                out=xfull[:, k, 0:CHW], in_=xT_t[:, k, 0:CHW])
            nc.scalar.dma_start(out=wq_sb[:, k, :], in_=wq_t[:, k, :])
            nc.scalar.dma_start(out=wk_sb[:, k, :], in_=wk_t[:, k, :])
        nc.scalar.dma_start(out=wv_sb, in_=wv_t)
        for ch in range(1, NCH):
            for k in range(KD):
                eng = nc.sync if (ch + k) % 2 == 0 else nc.scalar
                eng.dma_start(
                    out=xfull[:, k, ch * CHW:(ch + 1) * CHW],
                    in_=xT_t[:, k, ch * CHW:(ch + 1) * CHW])
        nc.gpsimd.dma_start(out=wo_sb, in_=wo_t)

        tensors = (qT, kT, v_sb, oT, trimask)

        f = _Fillers()

        def _proj_dr(which, pair_, ch):
            # which: 0=Q, 1=K; 4 DoubleRow matmuls + copy-out
            cell = {}
            w_sb = wq_sb if which == 0 else wk_sb
            dst = qT if which == 0 else kT

            def mm(t, cell=cell, ch=ch, w_sb=w_sb, pair_=pair_):
                if t == 0:
                    cell["p"] = ps_fill.tile(
                        [P, CHW], F32, tag="fill", name="fillqk")
                if USE_FP8:
                    nc.tensor.matmul(
                        cell["p"],
                        w_sb[:, 2 * t:2 * t + 2, pair_ * P:(pair_ + 1) * P],
                        xfull[:, 2 * t:2 * t + 2, ch * CHW:(ch + 1) * CHW],
                        start=(t == 0), stop=(t == KDH - 1),
                        perf_mode=DRMODE)
                else:
                    for u in (2 * t, 2 * t + 1):
                        nc.tensor.matmul(
                            cell["p"],
                            w_sb[:, u, pair_ * P:(pair_ + 1) * P],
                            xfull[:, u, ch * CHW:(ch + 1) * CHW],
                            start=(u == 0), stop=(u == KD - 1))

            def copy(cell=cell, ch=ch, dst=dst, pair_=pair_):
                nc.vector.tensor_copy(
                    out=dst[:, pair_, ch * CHW:(ch + 1) * CHW],
                    in_=cell["p"])

            for t in range(KDH):
                f.add(lambda t=t: mm(t))
            f.add(copy)

        def _v_dr(sb):
            cell = {}

            def mm(t, cell=cell, sb=sb):
                if t == 0:
                    cell["pv"] = ps_fill.tile(
                        [P, CHW], F32, tag="fill", name="fillpv")
                if USE_FP8:
                    nc.tensor.matmul(
                        cell["pv"][:, 0:GC],
                        xfull[:, 2 * t:2 * t + 2, sb * P:(sb + 1) * P],
                        wv_sb[:, 2 * t:2 * t + 2, :],
                        start=(t == 0), stop=(t == KDH - 1),
                        perf_mode=DRMODE)
                else:
                    for u in (2 * t, 2 * t + 1):
                        nc.tensor.matmul(
                            cell["pv"][:, 0:GC],
                            xfull[:, u, sb * P:(sb + 1) * P],
                            wv_sb[:, u, :],
                            start=(u == 0), stop=(u == KD - 1))

            def copy(cell=cell, sb=sb):
                nc.vector.tensor_copy(
                    out=v_sb[:, sb, :, 0:HD],
                    in_=cell["pv"][:, 0:GC].rearrange(
                        "p (h d) -> p h d", h=GH))

            for t in range(KDH):
                f.add(lambda t=t: mm(t))
            f.add(copy)

        # prerequisites in consumption order: per chunk, pair-0 Q/K + V
        # blocks, then pair-1 Q/K
        for ch in range(NCH):
            _proj_dr(0, 0, ch)
            _proj_dr(1, 0, ch)
            for s4 in range(CHW // P):
                _v_dr(ch * (CHW // P) + s4)
            f.add_marker(("pre0", ch))
            _proj_dr(0, 1, ch)
            _proj_dr(1, 1, ch)
            f.add_marker(("qk1", ch))

        def _outproj_chunk(c):
            for s4 in range(CHW // P):
                sb = c * (CHW // P) + s4
                cell = {}

                def alloc(cell=cell):
                    cell["ysb"] = ypool.tile(
                        [P, D], F32, tag="ysb", name="ysb")

                f.add(alloc)
                for nch in range(2):
                    def mm(gc, cell=cell, sb=sb, nch=nch):
                        if gc == 0:
                            cell["py"] = ps_fill.tile(
                                [P, CHW], F32, tag="fill", name="fillpy")
                        nc.tensor.matmul(
                            cell["py"], oT[:, gc, sb * P:(sb + 1) * P],
                            wo_sb[:, gc, nch * CHW:(nch + 1) * CHW],
                            start=(gc == 0), stop=(gc == 1))

                    def cp(cell=cell, nch=nch, c=c):
                        dst = cell["ysb"][:, nch * CHW:(nch + 1) * CHW]
                        if c == NCH - 1:
                            nc.scalar.copy(out=dst, in_=cell["py"])
                        else:
                            nc.vector.tensor_copy(out=dst, in_=cell["py"])

                    f.add(lambda mm=mm: mm(0))
                    f.add(lambda mm=mm: mm(1))
                    f.add(cp)

                def out_dma(cell=cell, sb=sb):
                    nc.sync.dma_start(
                        out=y[sb * P:(sb + 1) * P, :], in_=cell["ysb"])

                f.add(out_dma)

        for c in range(NCH):
            f.drain_until(("pre0", c))
            _emit_chunk_attention(tc, 0, c, pools, tensors, f, None)
            f.drain_until(("qk1", c))
            _emit_chunk_attention(tc, 1, c, pools, tensors, f, _outproj_chunk)
        f.drain()


def _fix_instruction_waits(nc):
    """Some lowered ISA structs (fp32r matmul LDW, DMA pseudo) carry at most
    one sync wait. Normalize: hoist excess waits onto NoOps inserted
    immediately before the instruction in the scheduled stream (same engine,
    so program order preserves the wait semantics)."""
    fixed = 0
    for blk in nc.m.functions[0].blocks:
        insts = blk.instructions
        idx = 0
        while idx < len(insts):
            inst = insts[idx]
            si = getattr(inst, "sync_info", None)
            if si is not None and len(si.on_wait) > 1:
                waits = list(si.on_wait)
                for j, wt in enumerate(waits[:-1]):
                    nop = mybir.InstNoOp(
                        name=f"I-wfix{fixed}-{j}-{inst.name}",
                        engine=inst.engine,
                        sync_info=mybir.SyncInfo(on_wait=[wt], on_update=[]))
                    insts.insert(idx, nop)
                    idx += 1
                inst.sync_info = mybir.SyncInfo(
                    on_wait=[waits[-1]], on_update=list(si.on_update))
                fixed += 1
            idx += 1
    return fixed


def _build():
    global _NC_CACHE
    if _NC_CACHE is None:
        nc = bass.Bass()
        with tile.TileContext(nc) as tc:
            _emit(tc)
        _fix_instruction_waits(nc)
        _NC_CACHE = nc
    return _NC_CACHE


def kernel(x, Wq, Wkv, Wo):
    global LAST_RESULTS
    x = np.asarray(x, dtype=np.float32)
    Wq = np.asarray(Wq, dtype=np.float32)
    Wkv = np.asarray(Wkv, dtype=np.float32)
    Wo = np.asarray(Wo, dtype=np.float32)

    nc = _build()
    bf = ml_dtypes.bfloat16
    xdt = ml_dtypes.float8_e4m3 if USE_FP8 else bf
    in_maps = []
    for c in range(8):
        b, g = divmod(c, 4)
        cs = slice(GC * g, GC * (g + 1))
        in_maps.append({
            "xT": np.ascontiguousarray(x[b].T).astype(xdt),
            "wq": np.ascontiguousarray(Wq[:, cs] * WSCALE).astype(xdt),
            "wk": np.ascontiguousarray(
                Wkv[:, 0:D][:, cs] * WSCALE).astype(xdt),
            "wv": np.ascontiguousarray(
                Wkv[:, D:2 * D][:, cs] * WSCALE).astype(xdt),
            "wo": np.ascontiguousarray(Wo[cs, :]).astype(bf),
        })

    trace = os.environ.get("ATTN_KERNEL_TRACE", "0") == "1"
    res = run_bass_kernel_spmd(nc, in_maps, list(range(8)), trace=trace)
    LAST_RESULTS = res

    out = np.zeros((B, S, D), dtype=np.float32)
    for c in range(8):
        b = c // 4
        out[b] += res.results[c]["y"]
    return out


if __name__ == "__main__":
    rng = np.random.default_rng(0)
    s = 1.0 / np.sqrt(D)
    inputs = {
        "x": rng.standard_normal((B, S, D), dtype=np.float32),
        "Wq": rng.standard_normal((D, D), dtype=np.float32) * s,
        "Wkv": rng.standard_normal((D, 2 * D), dtype=np.float32) * s,
        "Wo": rng.standard_normal((D, D), dtype=np.float32) * s,
    }
    out = kernel(**inputs)
    print("out", out.shape, out.dtype, float(np.abs(out).mean()))
